# revision 1
# baseline (speedup 1.0000x reference)
"""Bahdanau additive attention on 8 Trainium2 NeuronCores.

reference:
    eh = enc @ W_h.T            [B,S,H]
    qs = q   @ W_s.T            [B,T,H]
    score[b,t,s] = sum_h v[h] * tanh(eh[b,s,h] + qs[b,t,h])
    score = where(mask, score, -inf); attn = softmax_s(score)
    ctx[b,t,:] = sum_s attn[b,t,s] * enc[b,s,:]

Sharding: data-parallel over batch B=8, one batch per NeuronCore.

Per-core device program (all layouts chosen so the H-contraction of the
score sits on the partition axis):
  - inputs arrive pre-transposed from host: encT/qT [H,S|T], whT/wsT [H,H]
  - PE: ehT[d,s] = whT.T @ encT, qsT[d,t] = wsT.T @ qT   (PSUM, fp32)
  - main loop over s: DVE adds qsT + ehT[:,s] (per-partition scalar add),
    ACT computes tanh on large batched tiles (fp16 out),
    PE reduces over d via per-s stationary matvec with v -> score[t,s] PSUM
  - softmax over s on the free axis (DVE reduce_max / ACT exp+accum /
    DVE reciprocal), normalization folded into the context epilogue
  - PE transposes attn, context matmul against enc natural layout
"""

import sys

try:
    import concourse.bass as bass  # noqa: F401
except ImportError:  # pragma: no cover
    sys.path.insert(0, "/opt/trn_rl_repo")

import numpy as np

import concourse.bass as bass
import concourse.bacc as bacc
import concourse.mybir as mybir
from concourse import tile

FP32 = mybir.dt.float32
FP16 = mybir.dt.float16

N_CORES = 8
H = 512
T_FULL = 256
S_FULL = 256


def build_program(T=T_FULL, S=S_FULL, n_cores=N_CORES, nrep=1, debug=False, ablate=()):
    """Build the per-core Bass program. T/S parametrized for cheap sim runs."""
    assert H % 128 == 0 and T % 128 == 0 and S % 128 == 0
    DC = H // 128          # contraction chunks for the score reduction
    TB = T // 128          # t blocks (partition blocks of the score)
    SB = S // 128          # s blocks
    G = 32                 # s values batched per tanh activation

    nc = bacc.Bacc("TRN2", target_bir_lowering=False, debug=debug,
                   num_devices=n_cores)

    encT_d = nc.dram_tensor("encT", [H, S], FP32, kind="ExternalInput")
    enc_d = nc.dram_tensor("enc", [S, H], FP32, kind="ExternalInput")
    qT_d = nc.dram_tensor("qT", [H, T], FP32, kind="ExternalInput")
    whT_d = nc.dram_tensor("whT", [H, H], FP32, kind="ExternalInput")
    wsT_d = nc.dram_tensor("wsT", [H, H], FP32, kind="ExternalInput")
    v_d = nc.dram_tensor("v16", [128, DC], FP16, kind="ExternalInput")
    pen_d = nc.dram_tensor("pen", [1, S], FP32, kind="ExternalInput")
    ones_d = nc.dram_tensor("ones", [1, 128], FP32, kind="ExternalInput")
    ident_d = nc.dram_tensor("ident", [128, 128], FP32, kind="ExternalInput")
    ctx_d = nc.dram_tensor("ctx", [T, H], FP32, kind="ExternalOutput")

    with tile.TileContext(nc) as tc:
        with (
            tc.tile_pool(name="const", bufs=1) as const_pool,
            tc.tile_pool(name="proj", bufs=1) as proj_pool,
            tc.tile_pool(name="xadd", bufs=2) as xadd_pool,
            tc.tile_pool(name="xtanh", bufs=3) as xtanh_pool,
            tc.tile_pool(name="post", bufs=1) as post_pool,
            tc.tile_pool(name="ppsum", bufs=2, space=bass.MemorySpace.PSUM) as ppsum,
            tc.tile_pool(name="spsum", bufs=1, space=bass.MemorySpace.PSUM) as spsum,
            tc.tile_pool(name="apsum", bufs=2, space=bass.MemorySpace.PSUM) as apsum,
            tc.tile_pool(name="cpsum", bufs=2, space=bass.MemorySpace.PSUM) as cpsum,
        ):
            # ---- load constants / inputs (few big DMAs; critical first) ----
            whT_cat = const_pool.tile([128, 4 * H], FP32, name="whT_cat")
            wsT_cat = const_pool.tile([128, 4 * H], FP32, name="wsT_cat")
            encT_cat = const_pool.tile([128, DC * S], FP32, name="encT_cat")
            qT_cat = const_pool.tile([128, DC * T], FP32, name="qT_cat")
            enc_cat = const_pool.tile([128, SB * H], FP32, name="enc_cat")
            v_sb = const_pool.tile([128, DC], FP16, tag="v")
            pen_sb = const_pool.tile([1, S], FP32, tag="pen")
            ones_sb = const_pool.tile([1, 128], FP32, tag="ones")
            ident_sb = const_pool.tile([128, 128], FP32, tag="ident")

            def cat_load(dst, src_d, blocks, width):
                nc.sync.dma_start(
                    dst[:].rearrange("p (a j) -> p a j", a=blocks),
                    src_d.rearrange("(a p) j -> p a j", p=128),
                )

            def w_cols(dst, src_d, lo, hi):
                nc.sync.dma_start(
                    dst[:].rearrange("p (a j) -> p a j", a=4)[:, :, lo:hi],
                    src_d.rearrange("(a p) j -> p a j", p=128)[:, :, lo:hi],
                )

            w_cols(whT_cat, whT_d, 0, 128)
            cat_load(encT_cat, encT_d, DC, S)
            w_cols(wsT_cat, wsT_d, 0, 128)
            cat_load(qT_cat, qT_d, DC, T)
            w_cols(whT_cat, whT_d, 128, H)
            w_cols(wsT_cat, wsT_d, 128, H)
            nc.sync.dma_start(pen_sb[:], pen_d[:])
            nc.sync.dma_start(ones_sb[:], ones_d[:])
            nc.sync.dma_start(v_sb[:], v_d[:])
            cat_load(enc_cat, enc_d, SB, H)
            nc.sync.dma_start(ident_sb[:], ident_d[:])

            whT_sb = [whT_cat[:, H * i:H * (i + 1)] for i in range(4)]
            wsT_sb = [wsT_cat[:, H * i:H * (i + 1)] for i in range(4)]
            encT_sb = [encT_cat[:, S * i:S * (i + 1)] for i in range(DC)]
            qT_sb = [qT_cat[:, T * i:T * (i + 1)] for i in range(DC)]
            enc_sb = [enc_cat[:, H * i:H * (i + 1)] for i in range(SB)]

            import contextlib

            def _rep_ctx():
                if nrep == 1:
                    return contextlib.nullcontext()
                return tc.For_i(0, nrep, 1)

            with _rep_ctx():
                # ---- projections, emitted lazily (dc+1 prefetched during dc) ----
                ehT_sb = [post_pool.tile([128, S], FP32, name=f"ehT{i}", tag=f"ehT{i}") for i in range(DC)]
                qsT_sb = [post_pool.tile([128, T], FP16, name=f"qsT{i}", tag=f"qsT{i}") for i in range(DC)]

                def project(dc):
                    eh_ps = ppsum.tile([128, S], FP32, tag="proj_ps")
                    for hc in range(4):
                        nc.tensor.matmul(
                            eh_ps[:],
                            whT_sb[hc][:, 128 * dc:128 * (dc + 1)],
                            encT_sb[hc],
                            start=(hc == 0), stop=(hc == 3),
                        )
                    nc.vector.tensor_copy(ehT_sb[dc][:], eh_ps[:])
                    qs_ps = ppsum.tile([128, T], FP32, tag="proj_ps")
                    for hc in range(4):
                        nc.tensor.matmul(
                            qs_ps[:],
                            wsT_sb[hc][:, 128 * dc:128 * (dc + 1)],
                            qT_sb[hc],
                            start=(hc == 0), stop=(hc == 3),
                        )
                    nc.vector.tensor_copy(qsT_sb[dc][:], qs_ps[:])

                project(0)

                # ---- score: psum[t, s] accumulated column by column ----
                score_ps = [spsum.tile([128, S], FP32, name=f"score{tb}", tag=f"score{tb}")
                            for tb in range(TB)]
                for tb in range(TB):
                    # seed every column with the mask penalty (broadcast over t)
                    nc.tensor.matmul(
                        score_ps[tb][:], ones_sb[:], pen_sb[:],
                        start=True, stop=False, skip_group_check=True,
                    )

                FUSED0 = 16 if G >= 32 else 0

                def chunks_for(dc):
                    rem = S - (FUSED0 if dc == 0 else 0)
                    first = [G // 2, G // 2] if dc == 0 else []
                    last = [16, 8, 8] if (dc == DC - 1 and S >= 64) else []
                    mid_total = rem - sum(first) - sum(last)
                    assert mid_total >= 0
                    mids = [G] * (mid_total // G)
                    if mid_total % G:
                        mids.append(mid_total % G)
                    ws = first + mids + last
                    out, s0 = [], FUSED0 if dc == 0 else 0
                    for w in ws:
                        out.append((s0, w))
                        s0 += w
                    assert s0 == S
                    return out

                for dc in range(DC):
                    if dc == 0 and FUSED0:
                        # ACT-fused lead-in: tanh(qsT + ehT[:, s]) with no DVE
                        # dependency, so ACT starts while DVE builds its lead
                        xf = xtanh_pool.tile([128, FUSED0 * T], FP16, tag="xtanh")
                        for si in range(FUSED0):
                            nc.scalar.activation(
                                xf[:, si * T:(si + 1) * T], qsT_sb[0][:],
                                mybir.ActivationFunctionType.Tanh,
                                bias=ehT_sb[0][:, si:si + 1])
                        if "mm" not in ablate:
                            for si in range(FUSED0):
                                base = si * T
                                for tb in range(TB):
                                    nc.tensor.matmul(
                                        score_ps[tb][:, si:si + 1],
                                        xf[:, base + 128 * tb:base + 128 * (tb + 1)],
                                        v_sb[:, 0:1],
                                        start=False, stop=False,
                                        skip_group_check=True,
                                    )
                    for ci, (s0, w) in enumerate(chunks_for(dc)):
                        if ci == 1 and dc + 1 < DC:
                            project(dc + 1)
                        xadd = xadd_pool.tile([128, w * T], FP16, tag="xadd")
                        if "dve" not in ablate:
                            for si in range(w):
                                s = s0 + si
                                nc.vector.tensor_scalar_add(
                                    xadd[:, si * T:(si + 1) * T],
                                    qsT_sb[dc][:],
                                    ehT_sb[dc][:, s:s + 1],
                                )
                        xtanh = xtanh_pool.tile([128, w * T], FP16, tag="xtanh")
                        if "act" not in ablate:
                            if "dve" in ablate:
                                src_ap = (qsT_sb[dc][:]
                                          .rearrange("p (o t) -> p o t", o=1)
                                          .broadcast_to([128, w, T]))
                                nc.scalar.activation(
                                    xtanh[:].rearrange("p (o t) -> p o t", o=w),
                                    src_ap, mybir.ActivationFunctionType.Tanh)
                            else:
                                nc.scalar.activation(
                                    xtanh[:], xadd[:],
                                    mybir.ActivationFunctionType.Tanh)
                        if "mm" not in ablate:
                            for si in range(w):
                                s = s0 + si
                                base = si * T
                                for tb in range(TB):
                                    nc.tensor.matmul(
                                        score_ps[tb][:, s:s + 1],
                                        xtanh[:, base + 128 * tb:base + 128 * (tb + 1)],
                                        v_sb[:, dc:dc + 1],
                                        start=False, stop=(dc == DC - 1),
                                        skip_group_check=True,
                                    )

                # ---- softmax over s (free axis) ----
                attn_sb = [post_pool.tile([128, S], FP32, name=f"attn{tb}", tag=f"attn{tb}")
                           for tb in range(TB)]
                rden = [post_pool.tile([128, 1], FP32, name=f"rden{tb}", tag=f"rden{tb}")
                        for tb in range(TB)]
                for tb in range(TB):
                    nmax = post_pool.tile([128, 1], FP32, name=f"nmax{tb}", tag=f"nmax{tb}")
                    nc.vector.reduce_max(
                        nmax[:], score_ps[tb][:],
                        axis=mybir.AxisListType.X, negate=True)
                    den = post_pool.tile([128, 1], FP32, name=f"den{tb}", tag=f"den{tb}")
                    nc.scalar.activation(
                        attn_sb[tb][:], score_ps[tb][:],
                        mybir.ActivationFunctionType.Exp,
                        bias=nmax[:], scale=1.0, accum_out=den[:])
                    nc.vector.reciprocal(rden[tb][:], den[:])

                # ---- transpose attn -> attnT ----
                attnT_sb = [post_pool.tile([128, T], FP32, name=f"attnT{sb}", tag=f"attnT{sb}")
                            for sb in range(SB)]
                for sb in range(SB):
                    at_ps = apsum.tile([128, T], FP32, tag="at_ps")
                    for tb in range(TB):
                        nc.tensor.transpose(
                            at_ps[:, 128 * tb:128 * (tb + 1)],
                            attn_sb[tb][:, 128 * sb:128 * (sb + 1)],
                            ident_sb[:],
                        )
                    nc.vector.tensor_copy(attnT_sb[sb][:], at_ps[:])

                # ---- context: ctx[t, :] = sum_s attn[t,s] enc[s,:] (scaled) ----
                for tb in range(TB):
                    ctx_ps = cpsum.tile([128, H], FP32, tag="ctx_ps")
                    for sb in range(SB):
                        nc.tensor.matmul(
                            ctx_ps[:],
                            attnT_sb[sb][:, 128 * tb:128 * (tb + 1)],
                            enc_sb[sb],
                            start=(sb == 0), stop=(sb == SB - 1),
                        )
                    ctx_sb = post_pool.tile([128, H], FP32, name=f"ctx{tb}", tag=f"ctx{tb}")
                    nc.scalar.activation(
                        ctx_sb[:], ctx_ps[:],
                        mybir.ActivationFunctionType.Identity,
                        scale=rden[tb][:])
                    nc.sync.dma_start(ctx_d[128 * tb:128 * (tb + 1), :], ctx_sb[:])

    nc.compile()
    return nc


def make_in_maps(encoder_outputs, query, mask, W_h, W_s, v, T=T_FULL, S=S_FULL):
    B = encoder_outputs.shape[0]
    whT = np.ascontiguousarray(W_h.T.astype(np.float32))
    wsT = np.ascontiguousarray(W_s.T.astype(np.float32))
    v16 = np.ascontiguousarray(
        v.astype(np.float32).reshape(H // 128, 128).T.astype(np.float16))
    ones = np.ones((1, 128), np.float32)
    ident = np.eye(128, dtype=np.float32)
    in_maps = []
    for b in range(B):
        enc_b = np.ascontiguousarray(encoder_outputs[b].astype(np.float32))
        q_b = query[b].astype(np.float32)
        pen = np.where(mask[b], 0.0, -1e30).astype(np.float32).reshape(1, S)
        in_maps.append({
            "encT": np.ascontiguousarray(enc_b.T),
            "enc": enc_b,
            "qT": np.ascontiguousarray(q_b.T),
            "whT": whT,
            "wsT": wsT,
            "v16": v16,
            "pen": pen,
            "ones": ones,
            "ident": ident,
        })
    return in_maps


_PROGRAM_CACHE = {}


def kernel(encoder_outputs, query, mask, W_h, W_s, v):
    from concourse.bass_utils import run_bass_kernel_spmd

    B = encoder_outputs.shape[0]
    assert B == N_CORES
    key = (T_FULL, S_FULL, N_CORES)
    if key not in _PROGRAM_CACHE:
        _PROGRAM_CACHE[key] = build_program()
    nc = _PROGRAM_CACHE[key]
    in_maps = make_in_maps(encoder_outputs, query, mask, W_h, W_s, v)
    res = run_bass_kernel_spmd(nc, in_maps, list(range(N_CORES)))
    out = np.stack([res.results[b]["ctx"] for b in range(B)], axis=0)
    return out.astype(np.float32)



# revision 4
# speedup vs baseline: 6.4874x; 6.4874x over previous
"""Bahdanau additive attention on 8 Trainium2 NeuronCores.

reference:
    eh = enc @ W_h.T            [B,S,H]
    qs = q   @ W_s.T            [B,T,H]
    score[b,t,s] = sum_h v[h] * tanh(eh[b,s,h] + qs[b,t,h])
    score = where(mask, score, -inf); attn = softmax_s(score)
    ctx[b,t,:] = sum_s attn[b,t,s] * enc[b,s,:]

Sharding: data-parallel over batch B=8, one batch per NeuronCore.

Algorithm: the tanh over the [T,S,H] broadcast-sum is the dominant cost
(33.5M scalar-engine lookups/core ~ 218us). Instead we expand

    tanh(x) ~= alpha*x + sum_m c_m sin(m*w1*x)      (weighted LSQ fit)

and use sin(m*w1*(a+b)) = sin(m*w1*a)cos(m*w1*b) + cos(m*w1*a)sin(m*w1*b),
which turns the score into a plain matmul over an expanded contraction
dim (2M*H) that the PE array handles at full fp16 rate:

    score[t,s] = sum_m sum_h [v_h*sin_m(eh)][h,s] * [c_m*cos_m(qs)][h,t]
                            + [v_h*cos_m(eh)][h,s] * [c_m*sin_m(qs)][h,t]
                 + alpha * (v . eh)[s]   (+ t-only terms: softmax-invariant)

Per-core device program:
  - PE: ehT[d,s] = whT.T @ encT, qsT[d,t] = wsT.T @ qT  (fp16, PSUM fp32)
  - DVE: clamp projections to [-4.4, 4.4] (keeps all sin args in the
    fitted/periodic domain), cast fp16
  - ACT Sin: seed harmonics m=1,2 (args <= pi, where the HW spline is
    exact); DVE Chebyshev recurrence s_{m+1} = 2cos(w1 a) s_m - s_{m-1}
    for m=3..8 in fp16 2x mode (F-side seeds pre-scaled by v; linearity
    propagates the scale, G-side scaled by c_m per feature)
  - PE: 16 pair-matmuls accumulate score[t,s] in PSUM (+ mask penalty and
    the alpha*(v.eh) rank-1 term seeded via 1-row matmuls)
  - softmax over s on the free axis (DVE reduce_max / ACT exp+accum /
    DVE reciprocal), normalization folded into the context epilogue
  - PE transposes attn (fp16), context matmul against enc natural layout
"""

import sys

try:
    import concourse.bass as bass  # noqa: F401
except ImportError:  # pragma: no cover
    sys.path.insert(0, "/opt/trn_rl_repo")

import numpy as np

import concourse.bass as bass
import concourse.bacc as bacc
import concourse.mybir as mybir
from concourse import tile

FP32 = mybir.dt.float32
FP16 = mybir.dt.float16

N_CORES = 8
H = 512
T_FULL = 256
S_FULL = 256

# ---- tanh ~ alpha*x + sum_m cm sin(m*w1*x) fit (weighted LSQ) ----
M_HARM = 8
CLAMP = 4.4
W1 = np.pi / 9.0


def _fit_tanh_sine():
    xs = np.linspace(-2 * CLAMP, 2 * CLAMP, 8001)
    wgt = np.exp(-xs ** 2 / 4.0) + 1e-3
    A = np.concatenate(
        [xs[:, None], np.sin(np.outer(xs, np.arange(1, M_HARM + 1) * W1))], axis=1)
    Aw = A * wgt[:, None]
    coef = np.linalg.lstsq(Aw.T @ A, Aw.T @ np.tanh(xs), rcond=None)[0]
    return float(coef[0]), [float(c) for c in coef[1:]]


ALPHA, CM = _fit_tanh_sine()


def build_program(T=T_FULL, S=S_FULL, n_cores=N_CORES, nrep=1, debug=False, ablate=()):
    """Build the per-core Bass program. T/S parametrized for cheap sim runs."""
    assert H % 128 == 0 and T % 128 == 0 and S % 128 == 0
    DC = H // 128          # h chunks
    TB = T // 128          # t blocks (partition blocks of the score)
    SB = S // 128          # s blocks
    M = M_HARM

    nc = bacc.Bacc("TRN2", target_bir_lowering=False, debug=debug,
                   num_devices=n_cores)

    encT_d = nc.dram_tensor("encT", [H, S], FP16, kind="ExternalInput")
    enc_d = nc.dram_tensor("enc", [S, H], FP16, kind="ExternalInput")
    qT_d = nc.dram_tensor("qT", [H, T], FP16, kind="ExternalInput")
    whT_d = nc.dram_tensor("whT", [H, H], FP16, kind="ExternalInput")
    wsT_d = nc.dram_tensor("wsT", [H, H], FP16, kind="ExternalInput")
    vq_d = nc.dram_tensor("vq", [128, DC], FP32, kind="ExternalInput")
    av_d = nc.dram_tensor("av", [128, DC], FP16, kind="ExternalInput")
    pen_d = nc.dram_tensor("pen", [1, S], FP16, kind="ExternalInput")
    ones_d = nc.dram_tensor("ones", [1, 128], FP16, kind="ExternalInput")
    ident_d = nc.dram_tensor("ident", [128, 128], FP16, kind="ExternalInput")
    phase_d = nc.dram_tensor("phase", [128, 1], FP32, kind="ExternalInput")
    ctx_d = nc.dram_tensor("ctx", [T, H], FP32, kind="ExternalOutput")

    SIN = mybir.ActivationFunctionType.Sin
    EXP = mybir.ActivationFunctionType.Exp
    IDN = mybir.ActivationFunctionType.Identity

    with tile.TileContext(nc) as tc:
        with (
            tc.tile_pool(name="const", bufs=1) as const_pool,
            tc.tile_pool(name="work", bufs=1) as work_pool,
            tc.tile_pool(name="feat", bufs=1) as feat_pool,
            tc.tile_pool(name="tmp", bufs=4) as tmp_pool,
            tc.tile_pool(name="post", bufs=1) as post_pool,
            tc.tile_pool(name="ppsum", bufs=2, space=bass.MemorySpace.PSUM) as ppsum,
            tc.tile_pool(name="upsum", bufs=1, space=bass.MemorySpace.PSUM) as upsum,
            tc.tile_pool(name="spsum", bufs=1, space=bass.MemorySpace.PSUM) as spsum,
            tc.tile_pool(name="apsum", bufs=1, space=bass.MemorySpace.PSUM) as apsum,
            tc.tile_pool(name="cpsum", bufs=1, space=bass.MemorySpace.PSUM) as cpsum,
        ):
            # ---- load constants / inputs (few big DMAs; critical first) ----
            whT_cat = const_pool.tile([128, 4 * H], FP16, name="whT_cat")
            wsT_cat = const_pool.tile([128, 4 * H], FP16, name="wsT_cat")
            encT_cat = const_pool.tile([128, DC * S], FP16, name="encT_cat")
            qT_cat = const_pool.tile([128, DC * T], FP16, name="qT_cat")
            enc_cat = const_pool.tile([128, SB * H], FP16, name="enc_cat")
            vq_sb = const_pool.tile([128, DC], FP32, tag="vq")
            av_sb = const_pool.tile([128, DC], FP16, tag="av")
            pen_sb = const_pool.tile([1, S], FP16, tag="pen")
            ones_sb = const_pool.tile([1, 128], FP16, tag="ones")
            ident_sb = const_pool.tile([128, 128], FP16, tag="ident")
            phase_sb = const_pool.tile([128, 1], FP32, tag="phase")

            def cat_load(dst, src_d, blocks):
                nc.sync.dma_start(
                    dst[:].rearrange("p (a j) -> p a j", a=blocks),
                    src_d.rearrange("(a p) j -> p a j", p=128),
                )

            cat_load(whT_cat, whT_d, 4)
            cat_load(encT_cat, encT_d, DC)
            cat_load(wsT_cat, wsT_d, 4)
            cat_load(qT_cat, qT_d, DC)
            nc.sync.dma_start(vq_sb[:], vq_d[:])
            nc.sync.dma_start(av_sb[:], av_d[:])
            nc.sync.dma_start(pen_sb[:], pen_d[:])
            nc.sync.dma_start(ones_sb[:], ones_d[:])
            nc.sync.dma_start(phase_sb[:], phase_d[:])
            cat_load(enc_cat, enc_d, SB)
            nc.sync.dma_start(ident_sb[:], ident_d[:])

            whT_sb = [whT_cat[:, H * i:H * (i + 1)] for i in range(4)]
            wsT_sb = [wsT_cat[:, H * i:H * (i + 1)] for i in range(4)]
            encT_sb = [encT_cat[:, S * i:S * (i + 1)] for i in range(DC)]
            qT_sb = [qT_cat[:, T * i:T * (i + 1)] for i in range(DC)]
            enc_sb = [enc_cat[:, H * i:H * (i + 1)] for i in range(SB)]

            import contextlib

            def _rep_ctx():
                if nrep == 1:
                    return contextlib.nullcontext()
                return tc.For_i(0, nrep, 1)

            with _rep_ctx():
                # ---- projections -> clamped fp16 ehT/qsT [128, DC*{S,T}] ----
                ehT = work_pool.tile([128, DC * S], FP16, tag="ehT")
                qsT = work_pool.tile([128, DC * T], FP16, tag="qsT")

                def project(wT_sb, xT_sb, dst, N):
                    for dc in range(DC):
                        ps = ppsum.tile([128, N], FP32, tag="proj_ps")
                        for hc in range(4):
                            nc.tensor.matmul(
                                ps[:],
                                wT_sb[hc][:, 128 * dc:128 * (dc + 1)],
                                xT_sb[hc],
                                start=(hc == 0), stop=(hc == 3),
                            )
                        nc.vector.tensor_scalar(
                            dst[:, N * dc:N * (dc + 1)], ps[:],
                            CLAMP, -CLAMP,
                            mybir.AluOpType.min, mybir.AluOpType.max,
                        )

                project(whT_sb, encT_sb, ehT, S)
                project(wsT_sb, qT_sb, qsT, T)

                # ---- u[s] = alpha * (v . eh)[s]; upen = u + pen ----
                u_ps = upsum.tile([1, S], FP32, tag="u_ps")
                for dc in range(DC):
                    nc.tensor.matmul(
                        u_ps[:], av_sb[:, dc:dc + 1], ehT[:, S * dc:S * (dc + 1)],
                        start=(dc == 0), stop=(dc == DC - 1),
                    )
                upen = post_pool.tile([1, S], FP16, tag="upen")
                nc.vector.tensor_add(upen[:], u_ps[:], pen_sb[:])

                # ---- ACT seed harmonics m=1,2 (args <= pi) ----
                # raw (unscaled) seeds per side; F side gets v-scaled copies
                ehs = {}  # raw eh-side trig
                qss = {}  # raw qs-side trig
                for m in (1, 2):
                    for (d, nm) in ((ehs, "e"), (qss, "q")):
                        src = ehT if nm == "e" else qsT
                        s_t = feat_pool.tile([128, DC * S], FP16, tag=f"{nm}s{m}r")
                        c_t = feat_pool.tile([128, DC * S], FP16, tag=f"{nm}c{m}r")
                        nc.scalar.activation(s_t[:], src[:], SIN, scale=m * W1)
                        nc.scalar.activation(c_t[:], src[:], SIN, scale=m * W1,
                                             bias=phase_sb[:])
                        d[("s", m)] = s_t
                        d[("c", m)] = c_t

                # chain multipliers 2*cos(w1*a), unscaled
                Cch_e = feat_pool.tile([128, DC * S], FP16, tag="Cch_e")
                Cch_q = feat_pool.tile([128, DC * S], FP16, tag="Cch_q")
                nc.vector.tensor_scalar_mul(Cch_e[:], ehs[("c", 1)][:], 2.0)
                nc.vector.tensor_scalar_mul(Cch_q[:], qss[("c", 1)][:], 2.0)

                # ---- F chain seeds: v-scaled copies of eh seeds ----
                F = {}
                for m in (1, 2):
                    for t_ in ("s", "c"):
                        ft = feat_pool.tile([128, DC * S], FP16, tag=f"F{t_}{m}")
                        for dc in range(DC):
                            nc.vector.tensor_scalar_mul(
                                ft[:, S * dc:S * (dc + 1)],
                                ehs[(t_, m)][:, S * dc:S * (dc + 1)],
                                vq_sb[:, dc:dc + 1],
                            )
                        F[(t_, m)] = ft

                # ---- G scaled seed features: cm * qs seeds ----
                G = {}
                for m in (1, 2):
                    for t_ in ("s", "c"):
                        gt = feat_pool.tile([128, DC * T], FP16, tag=f"G{t_}{m}")
                        nc.vector.tensor_scalar_mul(
                            gt[:], qss[(t_, m)][:], CM[m - 1])
                        G[(t_, m)] = gt

                # ---- score PSUM seeds: pen+u broadcast over t rows ----
                score_ps = [spsum.tile([128, S], FP32, name=f"score{tb}",
                                       tag=f"score{tb}") for tb in range(TB)]
                for tb in range(TB):
                    nc.tensor.matmul(
                        score_ps[tb][:], ones_sb[:], upen[:],
                        start=True, stop=False, skip_group_check=True,
                    )

                def score_mm(m, last):
                    # score += Gc_m.T @ Fs_m + Gs_m.T @ Fc_m  (contraction h)
                    for pi, (gt, ft) in enumerate(
                            ((G[("c", m)], F[("s", m)]),
                             (G[("s", m)], F[("c", m)]))):
                        for hc in range(DC):
                            for tb in range(TB):
                                nc.tensor.matmul(
                                    score_ps[tb][:],
                                    gt[:, T * hc + 128 * tb:T * hc + 128 * (tb + 1)],
                                    ft[:, S * hc:S * (hc + 1)],
                                    start=False,
                                    stop=(last and pi == 1 and hc == DC - 1),
                                    skip_group_check=True,
                                )

                score_mm(1, False)
                score_mm(2, False)

                # ---- Chebyshev recurrence m=3..M + G scaling + matmuls ----
                qchain = {("s", 1): qss[("s", 1)], ("c", 1): qss[("c", 1)],
                          ("s", 2): qss[("s", 2)], ("c", 2): qss[("c", 2)]}
                for m in range(3, M + 1):
                    for t_ in ("s", "c"):
                        # F side: chain is v-scaled (linear recurrence)
                        ft = feat_pool.tile([128, DC * S], FP16, tag=f"F{t_}{m}")
                        tmp = tmp_pool.tile([128, DC * S], FP16, tag="rectmp")
                        nc.vector.tensor_mul(tmp[:], Cch_e[:], F[(t_, m - 1)][:])
                        nc.vector.tensor_sub(ft[:], tmp[:], F[(t_, m - 2)][:])
                        F[(t_, m)] = ft
                        # G side: raw chain + cm-scaled feature copy
                        qt = feat_pool.tile([128, DC * T], FP16, tag=f"q{t_}{m}r")
                        tmp2 = tmp_pool.tile([128, DC * T], FP16, tag="rectmp")
                        nc.vector.tensor_mul(tmp2[:], Cch_q[:], qchain[(t_, m - 1)][:])
                        nc.vector.tensor_sub(qt[:], tmp2[:], qchain[(t_, m - 2)][:])
                        qchain[(t_, m)] = qt
                        gt = feat_pool.tile([128, DC * T], FP16, tag=f"G{t_}{m}")
                        nc.vector.tensor_scalar_mul(gt[:], qt[:], CM[m - 1])
                        G[(t_, m)] = gt
                    score_mm(m, m == M)

                # ---- softmax over s (free axis) ----
                attn_sb = [post_pool.tile([128, S], FP16, name=f"attn{tb}",
                                          tag=f"attn{tb}") for tb in range(TB)]
                rden = [post_pool.tile([128, 1], FP32, name=f"rden{tb}",
                                       tag=f"rden{tb}") for tb in range(TB)]
                for tb in range(TB):
                    nmax = post_pool.tile([128, 1], FP32, tag=f"nmax{tb}")
                    nc.vector.reduce_max(
                        nmax[:], score_ps[tb][:],
                        axis=mybir.AxisListType.X, negate=True)
                    den = post_pool.tile([128, 1], FP32, tag=f"den{tb}")
                    nc.scalar.activation(
                        attn_sb[tb][:], score_ps[tb][:], EXP,
                        bias=nmax[:], scale=1.0, accum_out=den[:])
                    nc.vector.reciprocal(rden[tb][:], den[:])

                # ---- transpose attn -> attnT (fp16) ----
                attnT_sb = [post_pool.tile([128, T], FP16, name=f"attnT{sb}",
                                           tag=f"attnT{sb}") for sb in range(SB)]
                for sb in range(SB):
                    at_ps = apsum.tile([128, T], FP16, tag="at_ps")
                    for tb in range(TB):
                        nc.tensor.transpose(
                            at_ps[:, 128 * tb:128 * (tb + 1)],
                            attn_sb[tb][:, 128 * sb:128 * (sb + 1)],
                            ident_sb[:],
                        )
                    nc.vector.tensor_copy(attnT_sb[sb][:], at_ps[:])

                # ---- context: ctx[t, :] = sum_s attn[t,s] enc[s,:] (scaled) ----
                for tb in range(TB):
                    ctx_ps = cpsum.tile([128, H], FP32, tag="ctx_ps")
                    for sb in range(SB):
                        nc.tensor.matmul(
                            ctx_ps[:],
                            attnT_sb[sb][:, 128 * tb:128 * (tb + 1)],
                            enc_sb[sb],
                            start=(sb == 0), stop=(sb == SB - 1),
                        )
                    ctx_sb = post_pool.tile([128, H], FP32, tag=f"ctx{tb}")
                    nc.scalar.activation(
                        ctx_sb[:], ctx_ps[:], IDN, scale=rden[tb][:])
                    nc.sync.dma_start(ctx_d[128 * tb:128 * (tb + 1), :], ctx_sb[:])

    nc.compile()
    return nc


def make_in_maps(encoder_outputs, query, mask, W_h, W_s, v, T=T_FULL, S=S_FULL):
    B = encoder_outputs.shape[0]
    DC = H // 128
    whT = np.ascontiguousarray(W_h.astype(np.float32).T.astype(np.float16))
    wsT = np.ascontiguousarray(W_s.astype(np.float32).T.astype(np.float16))
    v32 = v.astype(np.float32)
    vq = np.ascontiguousarray(v32.reshape(DC, 128).T)                 # [128, DC] fp32
    av = np.ascontiguousarray((ALPHA * v32).reshape(DC, 128).T.astype(np.float16))
    ones = np.ones((1, 128), np.float16)
    ident = np.eye(128, dtype=np.float16)
    phase = np.full((128, 1), np.pi / 2, np.float32)
    in_maps = []
    for b in range(B):
        enc_b = np.ascontiguousarray(encoder_outputs[b].astype(np.float32))
        q_b = query[b].astype(np.float32)
        pen = np.where(mask[b], 0.0, -3.0e4).astype(np.float16).reshape(1, S)
        in_maps.append({
            "encT": np.ascontiguousarray(enc_b.T.astype(np.float16)),
            "enc": enc_b.astype(np.float16),
            "qT": np.ascontiguousarray(q_b.T.astype(np.float16)),
            "whT": whT,
            "wsT": wsT,
            "vq": vq,
            "av": av,
            "pen": pen,
            "ones": ones,
            "ident": ident,
            "phase": phase,
        })
    return in_maps


_PROGRAM_CACHE = {}


def kernel(encoder_outputs, query, mask, W_h, W_s, v):
    from concourse.bass_utils import run_bass_kernel_spmd

    B = encoder_outputs.shape[0]
    assert B == N_CORES
    key = (T_FULL, S_FULL, N_CORES)
    if key not in _PROGRAM_CACHE:
        _PROGRAM_CACHE[key] = build_program()
    nc = _PROGRAM_CACHE[key]
    in_maps = make_in_maps(encoder_outputs, query, mask, W_h, W_s, v)
    res = run_bass_kernel_spmd(nc, in_maps, list(range(N_CORES)))
    out = np.stack([res.results[b]["ctx"] for b in range(B)], axis=0)
    return out.astype(np.float32)
